# revision 65
# baseline (speedup 1.0000x reference)
"""TRN2 Bass kernel: fused multi-head attention (GPT-2 style, no causal mask).

Computes, for full inputs:
    qkv = X @ c_attn_w + c_attn_b ; q,k,v = split(qkv)
    per head: P = softmax(q k^T / sqrt(64)) ; a = P v
    out = merge_heads(a) @ c_proj_w + c_proj_b

Sharding: tensor-parallel over heads. 16 heads across 8 cores -> 2 heads/core.
Each core computes Q^T,K^T (d-major) and V (token-major, via PE transpose) for
its 2 heads, attention in score-transposed orientation (S^T = K Q^T per block,
so the softmax denominator falls out of the PE via an appended ones-column on
V), then a partial c_proj contribution (contraction over its 128 d-channels).
Partials are summed on the host (fp16 partials, fp32 host accumulate) - that
host sum is the tensor-parallel all-reduce.

Performance structure (v3): the PE p-state ramps to 2.4 GHz only while the
engine stays continuously busy, so the whole kernel is one near-gap-free PE
stream with every loop's engine budgets balanced (PE ~9.1us, ACT ~8.3us,
DVE ~8.8us per group-loop):
  - Both heads' S^T tiles land in one 2-bank PSUM tile so ScalarE exps
    [128,1024] per instruction (ACT runs exp back-to-back at ~1.03us/pair
    and does nothing else).
  - Loop g carries: P@V chain of group g-1 (front-loaded, steps 0-3, so
    its Z rows are ready early), the bcast+normalize of group g-2, the Z
    eviction + reciprocal chain of group g-1, and 8 c_proj matmuls of a
    finished batch half (steps 4-7).
  - The 1/Z Newton chain runs token-major: Z rows are DMA-reshaped
    [1,1024]->[128,8] so all 128 DVE lanes work (0.6us instead of 6.3us
    per group), then DMA'd back for the PE ones-column broadcast matmul.
  - P@V accumulators are evicted to SBUF right after the chain ends, so
    their 2 PSUM banks ring with just 2 tiles and the broadcast matmuls
    borrow the c_proj PSUM ring.
  - QKV projection for the last two column-chunks (qc 6,7) is deferred
    into attention loops 0-1 where it fills the PE while the exp pipeline
    ramps; their PSUM comes from the c_proj ring (q/k/v accumulate
    sequentially there).
"""

import os
from contextlib import ExitStack

import ml_dtypes
import numpy as np

import concourse.bass as bass
import concourse.mybir as mybir
from concourse import bacc, tile
from concourse.bass_utils import run_bass_kernel_spmd

F32 = mybir.dt.float32
F32R = mybir.dt.float32r
BF16 = mybir.dt.bfloat16
F16 = mybir.dt.float16

B, S, NX = 4, 1024, 1024
T = B * S  # 4096 tokens
NCORES = 8
HD = 64  # head dim
V2S = 2 * (HD + 1)  # 130 columns per 128-token block in the V2 layout
EXP = mybir.ActivationFunctionType.Exp
RECIP_MAGIC = 0x7EF311C3  # fp32 bit-trick reciprocal seed, ~5% rel err
I32 = mybir.dt.int32

_nc_cache = None


def _ensure_ntff_hook():
    """The agent image's `antenv` lacks `axon_hooks`, so bass_utils'
    trace=True path crashes on import. Synthesize the module and register
    the ctypes NTFF hook from trn_agent_boot (same thing boot() would have
    done). Returns True if profiling is available."""
    import sys
    import types

    try:
        from antenv.axon_hooks import get_axon_ntff_profile_hook  # noqa: F401

        return True
    except ImportError:
        pass
    try:
        import antenv
        from trn_agent_boot.trn_boot import _ntff_profile_via_ctypes

        mod = types.ModuleType("antenv.axon_hooks")
        mod._hook = _ntff_profile_via_ctypes("/opt/axon/libaxon_pjrt.so")

        def set_axon_ntff_profile_hook(h):
            mod._hook = h

        def get_axon_ntff_profile_hook():
            return mod._hook

        mod.set_axon_ntff_profile_hook = set_axon_ntff_profile_hook
        mod.get_axon_ntff_profile_hook = get_axon_ntff_profile_hook
        sys.modules["antenv.axon_hooks"] = mod
        antenv.axon_hooks = mod
        return True
    except Exception as e:  # pragma: no cover - profiling is best-effort
        print(f"kernel.py: NTFF profile hook unavailable ({e}); running untraced")
        return False


def _emit(nc, tc, xt, wq, wk, wv, wp, bq, bk, bv, identd, onecd, onefd, out):
    with ExitStack() as ctx:
        const = ctx.enter_context(tc.tile_pool(name="const", bufs=1))
        wq_sb = const.tile([128, 1024], BF16, tag="wq")
        wk_sb = const.tile([128, 1024], BF16, tag="wk")
        wv_sb = const.tile([128, 1024], BF16, tag="wv")
        wp_sb = const.tile([128, 1024], BF16, tag="wp")
        bq_sb = const.tile([128, 1], F32, tag="bq")
        bk_sb = const.tile([128, 1], F32, tag="bk")
        bv_sb = const.tile([128, 1], F32, tag="bv")
        ident = const.tile([128, 128], BF16, tag="ident")
        onec = const.tile([128, 64], BF16, tag="onec")
        onef = const.tile([128, 64], F32R, tag="onef")
        qt = const.tile([128, T], BF16, tag="qt")
        kt = const.tile([128, T], BF16, tag="kt")
        vt = const.tile([128, T], BF16, tag="vt")
        v2 = const.tile([128, (T // 128) * V2S], BF16, tag="v2")
        atall = const.tile([128, T], BF16, tag="atall")

        # NOTE: the ~16us before the first matmul is NEFF/semaphore init plus
        # the first weight/x loads; DMA-queue striping experiments did not
        # move it (the queues idle after ~4us - it's fixed framework cost).
        xtp = ctx.enter_context(tc.tile_pool(name="xtp", bufs=4))

        def load_xt(qc):
            c0 = qc * 512
            xt_t = xtp.tile([128, 8 * 512], BF16, name="xt_t")
            for k in range(8):
                nc.sync.dma_start(
                    xt_t[:, k * 512 : (k + 1) * 512],
                    xt[k * 128 : (k + 1) * 128, c0 : c0 + 512],
                )
            return xt_t

        nc.sync.dma_start(wq_sb[:], wq)
        nc.sync.dma_start(wk_sb[:], wk)
        nc.sync.dma_start(wv_sb[:], wv)
        nc.sync.dma_start(wp_sb[:], wp)
        nc.sync.dma_start(bq_sb[:], bq)
        nc.sync.dma_start(bk_sb[:], bk)
        nc.sync.dma_start(bv_sb[:], bv)
        nc.sync.dma_start(ident[:], identd)
        nc.sync.dma_start(onec[:], onecd)
        nc.sync.dma_start(onef[:], onefd)

        # ones columns of V2: per token block, col 64 (head 0) and col 129 (head 1)
        v2_ones = v2[:].rearrange("p (t h e) -> p t h e", h=2, e=HD + 1)[
            :, :, :, HD : HD + 1
        ]
        nc.vector.tensor_copy(
            v2_ones, onec[:].rearrange("p (t h e) -> p t h e", h=2, e=1)
        )

        # ---- Phase A: QKV projection (transposed: d-major) + V transposes ----
        # qc 0-5 run here; qc 6,7 are deferred into attention loops 0-1 (their
        # PSUM comes from the attention c_proj ring, q/k/v sequentially).
        # The previous qc's transposes + V2 copies are interleaved into the
        # middle of the current qc's matmul chain so the PE never waits on the
        # vt copy.
        with tc.tile_pool(name="psA", bufs=2, space="PSUM") as psA:
            def transpose_v(qc, pool):
                c0 = qc * 512
                tp = pool.tile([128, 512], BF16, tag="tp" if pool is psA else "opt",
                               name="tp")
                for t4 in range(4):
                    nc.tensor.transpose(
                        tp[:, t4 * 128 : (t4 + 1) * 128],
                        vt[:, c0 + t4 * 128 : c0 + (t4 + 1) * 128],
                        ident[:],
                    )
                src = tp[:].rearrange("p (t h e) -> p t h e", h=2, e=HD)
                dst = v2[:].rearrange("p (t h e) -> p t h e", h=2, e=HD + 1)[
                    :, qc * 4 : (qc + 1) * 4, :, 0:HD
                ]
                nc.vector.tensor_copy(dst, src)

            for qc in range(6):
                c0 = qc * 512
                xt_t = load_xt(qc)
                psq = psA.tile([128, 512], F32, tag="psq")
                psk = psA.tile([128, 512], F32, tag="psk")
                psv = psA.tile([128, 512], F32, tag="psv")
                for k in range(8):
                    ks = slice(k * 128, (k + 1) * 128)
                    rhs = xt_t[:, k * 512 : (k + 1) * 512]
                    first, last = k == 0, k == 7
                    nc.tensor.matmul(psq[:], wq_sb[:, ks], rhs, start=first, stop=last)
                    nc.tensor.matmul(psk[:], wk_sb[:, ks], rhs, start=first, stop=last)
                    nc.tensor.matmul(psv[:], wv_sb[:, ks], rhs, start=first, stop=last)
                    if k == 4 and qc > 0:
                        transpose_v(qc - 1, psA)
                nc.vector.tensor_scalar_add(qt[:, c0 : c0 + 512], psq[:], bq_sb[:, 0:1])
                nc.vector.tensor_scalar_add(kt[:, c0 : c0 + 512], psk[:], bk_sb[:, 0:1])
                nc.vector.tensor_scalar_add(vt[:, c0 : c0 + 512], psv[:], bv_sb[:, 0:1])
            transpose_v(5, psA)

        # ---- Attention + c_proj + deferred QKV: one interleaved PE stream --
        groups = [(b, q2) for b in range(4) for q2 in range(2)]
        with (
            tc.tile_pool(name="ptp", bufs=10) as ptp,
            tc.tile_pool(name="zrp", bufs=2) as zrp,
            tc.tile_pool(name="ztp", bufs=2) as ztp,
            tc.tile_pool(name="osp", bufs=4) as osp,
            tc.tile_pool(name="atp", bufs=2) as atp,
            tc.tile_pool(name="obp", bufs=24) as obp,
            tc.tile_pool(name="stp", bufs=2, space="PSUM") as stp,
            tc.tile_pool(name="otp", bufs=2, space="PSUM") as otp,
            tc.tile_pool(name="opj", bufs=2, space="PSUM") as opj,
        ):
            pts = {}   # (g, kb) -> pt tile [128,1024] bf16 (h0 cols 0:512)
            pots = {}  # g -> (ot0 tile, ot1 tile), both [128,512] f32

            def st_step(g, kb):
                b, q2 = groups[g]
                k0 = b * 1024 + kb * 128
                q0 = b * 1024 + q2 * 512
                stt = stp.tile([128, 1024], F32, tag="st")
                nc.tensor.matmul(
                    stt[:, 0:512],
                    kt[0:64, k0 : k0 + 128],
                    qt[0:64, q0 : q0 + 512],
                    start=True,
                    stop=True,
                )
                nc.tensor.matmul(
                    stt[:, 512:1024],
                    kt[64:128, k0 : k0 + 128],
                    qt[64:128, q0 : q0 + 512],
                    start=True,
                    stop=True,
                )
                pt = ptp.tile([128, 1024], BF16, tag="pt")
                nc.scalar.activation(pt[:], stt[:], EXP, scale=0.125)
                pts[(g, kb)] = pt

            def ot_step(g, kb):
                b, q2 = groups[g]
                if kb == 0:
                    ot0 = otp.tile([128, 512], F32, tag="ot")
                    ot1 = otp.tile([128, 512], F32, tag="ot")
                    pots[g] = (ot0, ot1)
                ot0, ot1 = pots[g]
                base = (b * 8 + kb) * V2S
                pt = pts[(g, kb)]
                first, last = kb == 0, kb == 7
                # per head: V cols then ones -> rows 0-63 data, row 64 = Z
                nc.tensor.matmul(
                    ot0[0:65, :],
                    v2[:, base : base + 65],
                    pt[:, 0:512],
                    start=first,
                    stop=last,
                )
                nc.tensor.matmul(
                    ot1[0:65, :],
                    v2[:, base + 65 : base + 130],
                    pt[:, 512:1024],
                    start=first,
                    stop=last,
                )
                if last:
                    del pts[(g, kb)]

            def finalize_evict(g):
                # Pull Z rows and the unnormalized A^T out of PSUM (DVE),
                # freeing the P@V accumulator banks, and kick off the
                # row->token-major reshape DMA for the Z rows.
                ot0, ot1 = pots.pop(g)
                zrow = zrp.tile([65, 1024], F32, tag="zrow")
                nc.vector.tensor_copy(zrow[64:65, 0:512], ot0[64:65, :])
                nc.vector.tensor_copy(zrow[64:65, 512:1024], ot1[64:65, :])
                os0 = osp.tile([64, 512], F32, tag="os", name="os0")
                nc.vector.tensor_copy(os0[:], ot0[0:64, :])
                os1 = osp.tile([64, 512], F32, tag="os", name="os1")
                nc.vector.tensor_copy(os1[:], ot1[0:64, :])
                zT = ztp.tile([128, 8], F32, tag="zt")
                nc.sync.dma_start(zT[:], zrow[64:65, :])
                return os0, os1, zT

            def finalize_chain(g, st8):
                # -1/Z on DVE, token-major so all 128 lanes work: bit-trick
                # seed + 2 Newton steps on [128,8], then DMA back to a row
                # for the PE broadcast. (vector.reciprocal is 3.3us on HW;
                # a row-major chain is 6.3us of DVE - this is ~0.6us.)
                _, _, zT = st8
                sd = ztp.tile([128, 8], I32, tag="sd")
                nc.vector.tensor_scalar(
                    sd[:],
                    zT[:].bitcast(I32),
                    -1,
                    RECIP_MAGIC,
                    mybir.AluOpType.mult,
                    mybir.AluOpType.add,
                )
                y0 = sd[:].bitcast(F32)
                t1 = ztp.tile([128, 8], F32, tag="t1")
                nc.vector.tensor_mul(t1[:], zT[:], y0)
                m1 = ztp.tile([128, 8], F32, tag="m1")  # m1 = -y1
                nc.vector.scalar_tensor_tensor(
                    m1[:], t1[:], 2.0, y0,
                    mybir.AluOpType.subtract, mybir.AluOpType.mult,
                )
                t2 = ztp.tile([128, 8], F32, tag="t2")  # t2 = -z*y1
                nc.vector.tensor_mul(t2[:], zT[:], m1[:])
                m2 = ztp.tile([128, 8], F32, tag="m2")  # m2 = -y2
                nc.vector.scalar_tensor_tensor(
                    m2[:], t2[:], 2.0, m1[:],
                    mybir.AluOpType.add, mybir.AluOpType.mult,
                )
                m2row = zrp.tile([65, 1024], F32R, tag="m2row")
                nc.sync.dma_start(m2row[64:65, :].bitcast(F32), m2[:])
                return m2row

            def finalize_bcast(g, st8, m2row):
                # Broadcast -1/Z across 64 partitions via the PE (negated
                # ones-column flips the sign back) into the c_proj PSUM
                # ring, then normalize (SBUF data x PSUM broadcast). h1
                # first: its path has the partition-shift DMA downstream.
                b, q2 = groups[g]
                q0 = b * 1024 + q2 * 512
                os0, os1, _ = st8
                bcb1 = opj.tile([64, 512], F32, tag="opt", name="bcb1")
                nc.tensor.matmul(
                    bcb1[:], onef[64:65, 0:64], m2row[64:65, 512:1024],
                    start=True, stop=True,
                )
                bcb0 = opj.tile([64, 512], F32, tag="opt", name="bcb0")
                nc.tensor.matmul(
                    bcb0[:], onef[64:65, 0:64], m2row[64:65, 0:512],
                    start=True, stop=True,
                )
                at1 = atp.tile([64, 512], BF16, tag="at1")
                nc.vector.tensor_mul(at1[:], os1[:], bcb1[:])
                nc.sync.dma_start(atall[64:128, q0 : q0 + 512], at1[:])
                nc.vector.tensor_mul(
                    atall[0:64, q0 : q0 + 512], os0[:], bcb0[:]
                )

            def cproj_half(b, q2, act_ocs=(0,), stripe=1):
                # 8 c_proj matmuls for one (batch, q2) half; PSUM evictions
                # for oc in act_ocs go to ACT, the rest to DVE. Each result
                # ships immediately, striped over 4 DMA queues, so no
                # output-DMA wall builds up at the end of the kernel.
                t0 = b * 1024 + q2 * 512
                for oc in range(8):
                    opt = opj.tile([128, 512], F32, tag="opt")
                    nc.tensor.matmul(
                        opt[:],
                        wp_sb[:, oc * 128 : (oc + 1) * 128],
                        atall[:, t0 : t0 + 512],
                        start=True,
                        stop=True,
                    )
                    ob = obp.tile([128, 512], F16, tag="ob", name="ob")
                    if oc in act_ocs:
                        nc.scalar.copy(ob[:], opt[:])
                    else:
                        nc.vector.tensor_copy(ob[:], opt[:])
                    rs = 128 // stripe
                    eng = nc.sync if stripe == 1 else nc.scalar
                    for s in range(stripe):
                        eng.dma_start(
                            out[oc * 128 + s * rs : oc * 128 + (s + 1) * rs,
                                t0 : t0 + 512],
                            ob[s * rs : (s + 1) * rs, :],
                        )
                    yield

            def qkv_chain(qc, which, xt_t):
                # deferred QKV for qc 6/7: one of q/k/v as a sequential
                # 8-matmul accumulation in the c_proj PSUM ring.
                c0 = qc * 512
                w_sb, b_sb, dstt = [
                    (wq_sb, bq_sb, qt), (wk_sb, bk_sb, kt), (wv_sb, bv_sb, vt)
                ][which]
                ps = opj.tile([128, 512], F32, tag="opt", name="ps")
                for k in range(8):
                    nc.tensor.matmul(
                        ps[:],
                        w_sb[:, k * 128 : (k + 1) * 128],
                        xt_t[:, k * 512 : (k + 1) * 512],
                        start=k == 0,
                        stop=k == 7,
                    )
                nc.vector.tensor_scalar_add(
                    dstt[:, c0 : c0 + 512], ps[:], b_sb[:, 0:1]
                )

            # Steady-state loop g carries: P@V of g-1 (steps 0-3, 2 pairs
            # per step), bcast+normalize of g-2 (step 1), Z/A^T eviction of
            # g-1 (step 3), recip chain of g-1 (step 5), c_proj half g-2
            # (steps 4-7). Loops 0-1 carry deferred QKV qc 6-7 instead.
            fz = {}   # g -> (os0, os1, zT) / m2row
            cg = None
            xt6 = xt7 = None
            for g in range(8):
                if g == 0:
                    xt6 = load_xt(6)
                    xt7 = load_xt(7)
                cg = (
                    iter(cproj_half(*divmod(g - 2, 2), act_ocs=(4,)))
                    if g >= 2
                    else None
                )
                for kb in range(8):
                    # steps 0-3: S^T first so its exp starts ~0.9us earlier
                    # per loop (the P@V pairs behind it keep the PE fed);
                    # steps 4-7: c_proj pieces first so a not-yet-ready S^T
                    # doesn't block them (PE executes in order).
                    if kb < 4:
                        st_step(g, kb)
                    if g >= 1 and kb < 4:
                        ot_step(g - 1, 2 * kb)
                        ot_step(g - 1, 2 * kb + 1)
                    if g >= 2 and kb == 1:
                        finalize_bcast(
                            g - 2, fz.pop(g - 2), fz.pop((g - 2, "m2"))
                        )
                    if cg is not None and kb >= 4:
                        next(cg)
                        next(cg)
                    if g == 0 and kb in (0, 1, 2):
                        qkv_chain(6, kb, xt6)
                    if g in (2, 3, 4) and kb == 2:
                        qkv_chain(7, g - 2, xt7)
                    if g == 1 and kb == 7:
                        transpose_v(6, opj)
                    if g == 5 and kb == 0:
                        transpose_v(7, opj)
                    if kb >= 4:
                        st_step(g, kb)
                    if g >= 1 and kb == 3:
                        fz[g - 1] = finalize_evict(g - 1)
                    if g >= 1 and kb == 5:
                        fz[(g - 1, "m2")] = finalize_chain(g - 1, fz[g - 1])
            # drain: P@V chain for group 7, then batch 3's two c_proj
            # halves. No exps remain, so ACT takes half the PSUM evictions
            # and the (3,0) half starts as soon as its normalize lands.
            st8 = m2row = None
            for kb in range(8):
                if kb < 4:
                    ot_step(7, 2 * kb)
                    ot_step(7, 2 * kb + 1)
                if kb == 0:
                    finalize_bcast(6, fz.pop(6), fz.pop((6, "m2")))
                if kb >= 1:
                    cg = iter(cproj_half(3, 0, (0, 2, 4, 6))) if kb == 1 else cg
                    next(cg)
                if kb == 4:
                    st8 = finalize_evict(7)
                if kb == 6:
                    m2row = finalize_chain(7, st8)
            for _ in cg:
                pass
            finalize_bcast(7, st8, m2row)
            for _ in cproj_half(3, 1, (0, 2, 4, 6)):
                pass


def _build_nc():
    nc = bacc.Bacc(
        "TRN2",
        target_bir_lowering=False,
        debug=False,
        enable_asserts=False,
        num_devices=NCORES,
    )
    xt = nc.dram_tensor("xt", [NX, T], BF16, kind="ExternalInput").ap()
    wq = nc.dram_tensor("wq", [128, 1024], BF16, kind="ExternalInput").ap()
    wk = nc.dram_tensor("wk", [128, 1024], BF16, kind="ExternalInput").ap()
    wv = nc.dram_tensor("wv", [128, 1024], BF16, kind="ExternalInput").ap()
    wp = nc.dram_tensor("wp", [128, 1024], BF16, kind="ExternalInput").ap()
    bq = nc.dram_tensor("bq", [128, 1], F32, kind="ExternalInput").ap()
    bk = nc.dram_tensor("bk", [128, 1], F32, kind="ExternalInput").ap()
    bv = nc.dram_tensor("bv", [128, 1], F32, kind="ExternalInput").ap()
    identd = nc.dram_tensor("ident", [128, 128], BF16, kind="ExternalInput").ap()
    onecd = nc.dram_tensor("onec", [128, 64], BF16, kind="ExternalInput").ap()
    onefd = nc.dram_tensor("onef", [128, 64], F32R, kind="ExternalInput").ap()
    out = nc.dram_tensor("out_t", [NX, T], F16, kind="ExternalOutput").ap()
    with tile.TileContext(nc) as tc:
        _emit(nc, tc, xt, wq, wk, wv, wp, bq, bk, bv, identd, onecd, onefd, out)
    nc.compile()
    return nc


def _pack_w(wcols):
    # [1024, 128] -> [128, 8*128] bf16: sbuf[p, k*128 + j] = W[k*128 + p, j]
    w = np.ascontiguousarray(np.asarray(wcols, dtype=np.float32))
    return np.ascontiguousarray(
        w.reshape(8, 128, 128).transpose(1, 0, 2).reshape(128, 1024)
    ).astype(ml_dtypes.bfloat16)


def kernel(hidden_states, c_attn_w, c_attn_b, c_proj_w, c_proj_b):
    global _nc_cache
    hidden_states = np.asarray(hidden_states, dtype=np.float32)
    c_attn_w = np.asarray(c_attn_w, dtype=np.float32)
    c_attn_b = np.asarray(c_attn_b, dtype=np.float32)
    c_proj_w = np.asarray(c_proj_w, dtype=np.float32)
    c_proj_b = np.asarray(c_proj_b, dtype=np.float32)

    if _nc_cache is None:
        _nc_cache = _build_nc()
    nc = _nc_cache

    X = hidden_states.reshape(T, NX)
    xt_np = np.ascontiguousarray(X.T).astype(ml_dtypes.bfloat16)

    in_maps = []
    for c in range(NCORES):
        cs = slice(c * 128, (c + 1) * 128)
        in_maps.append(
            {
                "xt": xt_np,
                "wq": _pack_w(c_attn_w[:, c * 128 : (c + 1) * 128]),
                "wk": _pack_w(c_attn_w[:, 1024 + c * 128 : 1024 + (c + 1) * 128]),
                "wv": _pack_w(c_attn_w[:, 2048 + c * 128 : 2048 + (c + 1) * 128]),
                "wp": np.ascontiguousarray(c_proj_w[cs, :]).astype(ml_dtypes.bfloat16),
                "bq": np.ascontiguousarray(c_attn_b[cs].reshape(128, 1)),
                "bk": np.ascontiguousarray(
                    c_attn_b[1024 + c * 128 : 1024 + (c + 1) * 128].reshape(128, 1)
                ),
                "bv": np.ascontiguousarray(
                    c_attn_b[2048 + c * 128 : 2048 + (c + 1) * 128].reshape(128, 1)
                ),
                "ident": np.eye(128, dtype=np.float32).astype(ml_dtypes.bfloat16),
                "onec": np.ones((128, 64), dtype=ml_dtypes.bfloat16),
                "onef": np.full((128, 64), -1.0, dtype=np.float32),
            }
        )

    trace = bool(int(os.environ.get("KERNEL_PROFILE", "0")))
    if trace:
        trace = _ensure_ntff_hook()
    try:
        res = run_bass_kernel_spmd(
            nc, in_maps, core_ids=list(range(NCORES)), trace=trace
        )
    except Exception:
        if not trace:
            raise
        print("kernel.py: traced run failed; retrying untraced")
        res = run_bass_kernel_spmd(nc, in_maps, core_ids=list(range(NCORES)))

    total = np.zeros((NX, T), np.float32)
    for r in res.results:
        total += r["out_t"].astype(np.float32)
    out = total.T.reshape(B, S, NX) + c_proj_b[None, None, :]
    kernel.last_exec_time_ns = res.exec_time_ns
    return out.astype(np.float32)


# revision 66
# speedup vs baseline: 1.0278x; 1.0278x over previous
"""TRN2 Bass kernel: fused multi-head attention (GPT-2 style, no causal mask).

Computes, for full inputs:
    qkv = X @ c_attn_w + c_attn_b ; q,k,v = split(qkv)
    per head: P = softmax(q k^T / sqrt(64)) ; a = P v
    out = merge_heads(a) @ c_proj_w + c_proj_b

Sharding: tensor-parallel over heads. 16 heads across 8 cores -> 2 heads/core.
Each core computes Q^T,K^T (d-major) and V (token-major, via PE transpose) for
its 2 heads, attention in score-transposed orientation (S^T = K Q^T per block,
so the softmax denominator falls out of the PE via an appended ones-column on
V), then a partial c_proj contribution (contraction over its 128 d-channels).
Partials are summed on the host (fp16 partials, fp32 host accumulate) - that
host sum is the tensor-parallel all-reduce.

Performance structure (v3): the PE p-state ramps to 2.4 GHz only while the
engine stays continuously busy, so the whole kernel is one near-gap-free PE
stream with every loop's engine budgets balanced (PE ~9.1us, ACT ~8.3us,
DVE ~8.8us per group-loop):
  - Both heads' S^T tiles land in one 2-bank PSUM tile so ScalarE exps
    [128,1024] per instruction (ACT runs exp back-to-back at ~1.03us/pair
    and does nothing else).
  - Loop g carries: P@V chain of group g-1 (front-loaded, steps 0-3, so
    its Z rows are ready early), the bcast+normalize of group g-2, the Z
    eviction + reciprocal chain of group g-1, and 8 c_proj matmuls of a
    finished batch half (steps 4-7).
  - The 1/Z Newton chain runs token-major: Z rows are DMA-reshaped
    [1,1024]->[128,8] so all 128 DVE lanes work (0.6us instead of 6.3us
    per group), then DMA'd back for the PE ones-column broadcast matmul.
  - P@V accumulators are evicted to SBUF right after the chain ends, so
    their 2 PSUM banks ring with just 2 tiles and the broadcast matmuls
    borrow the c_proj PSUM ring.
  - QKV projection for the last two column-chunks (qc 6,7) is deferred
    into attention loops 0-1 where it fills the PE while the exp pipeline
    ramps; their PSUM comes from the c_proj ring (q/k/v accumulate
    sequentially there).
"""

import os
from contextlib import ExitStack

import ml_dtypes
import numpy as np

import concourse.bass as bass
import concourse.mybir as mybir
from concourse import bacc, tile
from concourse.bass_utils import run_bass_kernel_spmd

F32 = mybir.dt.float32
F32R = mybir.dt.float32r
BF16 = mybir.dt.bfloat16
F16 = mybir.dt.float16

B, S, NX = 4, 1024, 1024
T = B * S  # 4096 tokens
NCORES = 8
HD = 64  # head dim
V2S = 2 * (HD + 1)  # 130 columns per 128-token block in the V2 layout
EXP = mybir.ActivationFunctionType.Exp
RECIP_MAGIC = 0x7EF311C3  # fp32 bit-trick reciprocal seed, ~5% rel err
I32 = mybir.dt.int32

_nc_cache = None


def _ensure_ntff_hook():
    """The agent image's `antenv` lacks `axon_hooks`, so bass_utils'
    trace=True path crashes on import. Synthesize the module and register
    the ctypes NTFF hook from trn_agent_boot (same thing boot() would have
    done). Returns True if profiling is available."""
    import sys
    import types

    try:
        from antenv.axon_hooks import get_axon_ntff_profile_hook  # noqa: F401

        return True
    except ImportError:
        pass
    try:
        import antenv
        from trn_agent_boot.trn_boot import _ntff_profile_via_ctypes

        mod = types.ModuleType("antenv.axon_hooks")
        mod._hook = _ntff_profile_via_ctypes("/opt/axon/libaxon_pjrt.so")

        def set_axon_ntff_profile_hook(h):
            mod._hook = h

        def get_axon_ntff_profile_hook():
            return mod._hook

        mod.set_axon_ntff_profile_hook = set_axon_ntff_profile_hook
        mod.get_axon_ntff_profile_hook = get_axon_ntff_profile_hook
        sys.modules["antenv.axon_hooks"] = mod
        antenv.axon_hooks = mod
        return True
    except Exception as e:  # pragma: no cover - profiling is best-effort
        print(f"kernel.py: NTFF profile hook unavailable ({e}); running untraced")
        return False


def _emit(nc, tc, xt, wq, wk, wv, wp, bq, bk, bv, identd, onecd, onefd, out):
    with ExitStack() as ctx:
        const = ctx.enter_context(tc.tile_pool(name="const", bufs=1))
        wq_sb = const.tile([128, 1024], BF16, tag="wq")
        wk_sb = const.tile([128, 1024], BF16, tag="wk")
        wv_sb = const.tile([128, 1024], BF16, tag="wv")
        wp_sb = const.tile([128, 1024], BF16, tag="wp")
        bq_sb = const.tile([128, 1], F32, tag="bq")
        bk_sb = const.tile([128, 1], F32, tag="bk")
        bv_sb = const.tile([128, 1], F32, tag="bv")
        ident = const.tile([128, 128], BF16, tag="ident")
        onec = const.tile([128, 64], BF16, tag="onec")
        onef = const.tile([128, 64], F32R, tag="onef")
        qt = const.tile([128, T], BF16, tag="qt")
        kt = const.tile([128, T], BF16, tag="kt")
        vt = const.tile([128, T], BF16, tag="vt")
        v2 = const.tile([128, (T // 128) * V2S], BF16, tag="v2")
        atall = const.tile([128, T], BF16, tag="atall")

        # NOTE: the ~16us before the first matmul is NEFF/semaphore init plus
        # the first weight/x loads; DMA-queue striping experiments did not
        # move it (the queues idle after ~4us - it's fixed framework cost).
        xtp = ctx.enter_context(tc.tile_pool(name="xtp", bufs=4))

        def load_xt(qc):
            c0 = qc * 512
            xt_t = xtp.tile([128, 8 * 512], BF16, name="xt_t")
            for k in range(8):
                nc.sync.dma_start(
                    xt_t[:, k * 512 : (k + 1) * 512],
                    xt[k * 128 : (k + 1) * 128, c0 : c0 + 512],
                )
            return xt_t

        nc.sync.dma_start(wq_sb[:], wq)
        nc.sync.dma_start(wk_sb[:], wk)
        nc.sync.dma_start(wv_sb[:], wv)
        nc.sync.dma_start(wp_sb[:], wp)
        nc.sync.dma_start(bq_sb[:], bq)
        nc.sync.dma_start(bk_sb[:], bk)
        nc.sync.dma_start(bv_sb[:], bv)
        nc.sync.dma_start(ident[:], identd)
        nc.sync.dma_start(onec[:], onecd)
        nc.sync.dma_start(onef[:], onefd)

        # ones columns of V2: per token block, col 64 (head 0) and col 129 (head 1)
        v2_ones = v2[:].rearrange("p (t h e) -> p t h e", h=2, e=HD + 1)[
            :, :, :, HD : HD + 1
        ]
        nc.vector.tensor_copy(
            v2_ones, onec[:].rearrange("p (t h e) -> p t h e", h=2, e=1)
        )

        # ---- Phase A: QKV projection (transposed: d-major) + V transposes ----
        # qc 0-5 run here; qc 6,7 are deferred into attention loops 0-1 (their
        # PSUM comes from the attention c_proj ring, q/k/v sequentially).
        # The previous qc's transposes + V2 copies are interleaved into the
        # middle of the current qc's matmul chain so the PE never waits on the
        # vt copy.
        with tc.tile_pool(name="psA", bufs=2, space="PSUM") as psA:
            def transpose_v(qc, pool):
                c0 = qc * 512
                tp = pool.tile([128, 512], BF16, tag="tp" if pool is psA else "opt",
                               name="tp")
                for t4 in range(4):
                    nc.tensor.transpose(
                        tp[:, t4 * 128 : (t4 + 1) * 128],
                        vt[:, c0 + t4 * 128 : c0 + (t4 + 1) * 128],
                        ident[:],
                    )
                src = tp[:].rearrange("p (t h e) -> p t h e", h=2, e=HD)
                dst = v2[:].rearrange("p (t h e) -> p t h e", h=2, e=HD + 1)[
                    :, qc * 4 : (qc + 1) * 4, :, 0:HD
                ]
                nc.vector.tensor_copy(dst, src)

            for qc in range(6):
                c0 = qc * 512
                xt_t = load_xt(qc)
                psq = psA.tile([128, 512], F32, tag="psq")
                psk = psA.tile([128, 512], F32, tag="psk")
                psv = psA.tile([128, 512], F32, tag="psv")
                for k in range(8):
                    ks = slice(k * 128, (k + 1) * 128)
                    rhs = xt_t[:, k * 512 : (k + 1) * 512]
                    first, last = k == 0, k == 7
                    nc.tensor.matmul(psq[:], wq_sb[:, ks], rhs, start=first, stop=last)
                    nc.tensor.matmul(psk[:], wk_sb[:, ks], rhs, start=first, stop=last)
                    nc.tensor.matmul(psv[:], wv_sb[:, ks], rhs, start=first, stop=last)
                    if k == 4 and qc > 0:
                        transpose_v(qc - 1, psA)
                nc.vector.tensor_scalar_add(qt[:, c0 : c0 + 512], psq[:], bq_sb[:, 0:1])
                nc.vector.tensor_scalar_add(kt[:, c0 : c0 + 512], psk[:], bk_sb[:, 0:1])
                nc.vector.tensor_scalar_add(vt[:, c0 : c0 + 512], psv[:], bv_sb[:, 0:1])
            transpose_v(5, psA)

        # ---- Attention + c_proj + deferred QKV: one interleaved PE stream --
        groups = [(b, q2) for b in range(4) for q2 in range(2)]
        with (
            tc.tile_pool(name="ptp", bufs=13) as ptp,
            tc.tile_pool(name="zrp", bufs=2) as zrp,
            tc.tile_pool(name="ztp", bufs=2) as ztp,
            tc.tile_pool(name="osp", bufs=4) as osp,
            tc.tile_pool(name="atp", bufs=2) as atp,
            tc.tile_pool(name="obp", bufs=24) as obp,
            tc.tile_pool(name="stp", bufs=2, space="PSUM") as stp,
            tc.tile_pool(name="otp", bufs=2, space="PSUM") as otp,
            tc.tile_pool(name="opj", bufs=2, space="PSUM") as opj,
        ):
            pts = {}   # (g, kb) -> pt tile [128,1024] bf16 (h0 cols 0:512)
            pots = {}  # g -> (ot0 tile, ot1 tile), both [128,512] f32

            def st_step(g, kb):
                b, q2 = groups[g]
                k0 = b * 1024 + kb * 128
                q0 = b * 1024 + q2 * 512
                stt = stp.tile([128, 1024], F32, tag="st")
                nc.tensor.matmul(
                    stt[:, 0:512],
                    kt[0:64, k0 : k0 + 128],
                    qt[0:64, q0 : q0 + 512],
                    start=True,
                    stop=True,
                )
                nc.tensor.matmul(
                    stt[:, 512:1024],
                    kt[64:128, k0 : k0 + 128],
                    qt[64:128, q0 : q0 + 512],
                    start=True,
                    stop=True,
                )
                pt = ptp.tile([128, 1024], BF16, tag="pt")
                nc.scalar.activation(pt[:], stt[:], EXP, scale=0.125)
                pts[(g, kb)] = pt

            def ot_step(g, kb):
                b, q2 = groups[g]
                if kb == 0:
                    ot0 = otp.tile([128, 512], F32, tag="ot")
                    ot1 = otp.tile([128, 512], F32, tag="ot")
                    pots[g] = (ot0, ot1)
                ot0, ot1 = pots[g]
                base = (b * 8 + kb) * V2S
                pt = pts[(g, kb)]
                first, last = kb == 0, kb == 7
                # per head: V cols then ones -> rows 0-63 data, row 64 = Z
                nc.tensor.matmul(
                    ot0[0:65, :],
                    v2[:, base : base + 65],
                    pt[:, 0:512],
                    start=first,
                    stop=last,
                )
                nc.tensor.matmul(
                    ot1[0:65, :],
                    v2[:, base + 65 : base + 130],
                    pt[:, 512:1024],
                    start=first,
                    stop=last,
                )
                if last:
                    del pts[(g, kb)]

            def finalize_evict(g):
                # Pull Z rows and the unnormalized A^T out of PSUM (DVE),
                # freeing the P@V accumulator banks, and kick off the
                # row->token-major reshape DMA for the Z rows.
                ot0, ot1 = pots.pop(g)
                zrow = zrp.tile([65, 1024], F32, tag="zrow")
                nc.vector.tensor_copy(zrow[64:65, 0:512], ot0[64:65, :])
                nc.vector.tensor_copy(zrow[64:65, 512:1024], ot1[64:65, :])
                os0 = osp.tile([64, 512], F32, tag="os", name="os0")
                nc.vector.tensor_copy(os0[:], ot0[0:64, :])
                os1 = osp.tile([64, 512], F32, tag="os", name="os1")
                nc.vector.tensor_copy(os1[:], ot1[0:64, :])
                zT = ztp.tile([128, 8], F32, tag="zt")
                nc.sync.dma_start(zT[:], zrow[64:65, :])
                return os0, os1, zT

            def finalize_chain(g, st8):
                # -1/Z on DVE, token-major so all 128 lanes work: bit-trick
                # seed + 2 Newton steps on [128,8], then DMA back to a row
                # for the PE broadcast. (vector.reciprocal is 3.3us on HW;
                # a row-major chain is 6.3us of DVE - this is ~0.6us.)
                _, _, zT = st8
                sd = ztp.tile([128, 8], I32, tag="sd")
                nc.vector.tensor_scalar(
                    sd[:],
                    zT[:].bitcast(I32),
                    -1,
                    RECIP_MAGIC,
                    mybir.AluOpType.mult,
                    mybir.AluOpType.add,
                )
                y0 = sd[:].bitcast(F32)
                t1 = ztp.tile([128, 8], F32, tag="t1")
                nc.vector.tensor_mul(t1[:], zT[:], y0)
                m1 = ztp.tile([128, 8], F32, tag="m1")  # m1 = -y1
                nc.vector.scalar_tensor_tensor(
                    m1[:], t1[:], 2.0, y0,
                    mybir.AluOpType.subtract, mybir.AluOpType.mult,
                )
                t2 = ztp.tile([128, 8], F32, tag="t2")  # t2 = -z*y1
                nc.vector.tensor_mul(t2[:], zT[:], m1[:])
                m2 = ztp.tile([128, 8], F32, tag="m2")  # m2 = -y2
                nc.vector.scalar_tensor_tensor(
                    m2[:], t2[:], 2.0, m1[:],
                    mybir.AluOpType.add, mybir.AluOpType.mult,
                )
                m2row = zrp.tile([65, 1024], F32R, tag="m2row")
                nc.sync.dma_start(m2row[64:65, :].bitcast(F32), m2[:])
                return m2row

            def finalize_bcast(g, st8, m2row):
                # Broadcast -1/Z across 64 partitions via the PE (negated
                # ones-column flips the sign back) into the c_proj PSUM
                # ring, then normalize (SBUF data x PSUM broadcast). h1
                # first: its path has the partition-shift DMA downstream.
                b, q2 = groups[g]
                q0 = b * 1024 + q2 * 512
                os0, os1, _ = st8
                bcb1 = opj.tile([64, 512], F32, tag="opt", name="bcb1")
                nc.tensor.matmul(
                    bcb1[:], onef[64:65, 0:64], m2row[64:65, 512:1024],
                    start=True, stop=True,
                )
                bcb0 = opj.tile([64, 512], F32, tag="opt", name="bcb0")
                nc.tensor.matmul(
                    bcb0[:], onef[64:65, 0:64], m2row[64:65, 0:512],
                    start=True, stop=True,
                )
                at1 = atp.tile([64, 512], BF16, tag="at1")
                nc.vector.tensor_mul(at1[:], os1[:], bcb1[:])
                nc.sync.dma_start(atall[64:128, q0 : q0 + 512], at1[:])
                nc.vector.tensor_mul(
                    atall[0:64, q0 : q0 + 512], os0[:], bcb0[:]
                )

            def cproj_half(b, q2, act_ocs=(0,), stripe=1):
                # 8 c_proj matmuls for one (batch, q2) half; PSUM evictions
                # for oc in act_ocs go to ACT, the rest to DVE. Each result
                # ships immediately, striped over 4 DMA queues, so no
                # output-DMA wall builds up at the end of the kernel.
                t0 = b * 1024 + q2 * 512
                for oc in range(8):
                    opt = opj.tile([128, 512], F32, tag="opt")
                    nc.tensor.matmul(
                        opt[:],
                        wp_sb[:, oc * 128 : (oc + 1) * 128],
                        atall[:, t0 : t0 + 512],
                        start=True,
                        stop=True,
                    )
                    ob = obp.tile([128, 512], F16, tag="ob", name="ob")
                    if oc in act_ocs:
                        nc.scalar.copy(ob[:], opt[:])
                    else:
                        nc.vector.tensor_copy(ob[:], opt[:])
                    rs = 128 // stripe
                    eng = nc.sync if stripe == 1 else nc.scalar
                    for s in range(stripe):
                        eng.dma_start(
                            out[oc * 128 + s * rs : oc * 128 + (s + 1) * rs,
                                t0 : t0 + 512],
                            ob[s * rs : (s + 1) * rs, :],
                        )
                    yield

            def qkv_chain(qc, which, xt_t):
                # deferred QKV for qc 6/7: one of q/k/v as a sequential
                # 8-matmul accumulation in the c_proj PSUM ring.
                c0 = qc * 512
                w_sb, b_sb, dstt = [
                    (wq_sb, bq_sb, qt), (wk_sb, bk_sb, kt), (wv_sb, bv_sb, vt)
                ][which]
                ps = opj.tile([128, 512], F32, tag="opt", name="ps")
                for k in range(8):
                    nc.tensor.matmul(
                        ps[:],
                        w_sb[:, k * 128 : (k + 1) * 128],
                        xt_t[:, k * 512 : (k + 1) * 512],
                        start=k == 0,
                        stop=k == 7,
                    )
                nc.vector.tensor_scalar_add(
                    dstt[:, c0 : c0 + 512], ps[:], b_sb[:, 0:1]
                )

            # Steady-state loop g carries: P@V of g-1 (steps 0-3, 2 pairs
            # per step), bcast+normalize of g-2 (step 1), Z/A^T eviction of
            # g-1 (step 3), recip chain of g-1 (step 5), c_proj half g-2
            # (steps 4-7). Loops 0-1 carry deferred QKV qc 6-7 instead.
            fz = {}   # g -> (os0, os1, zT) / m2row
            cg = None
            xt6 = xt7 = None
            for g in range(8):
                if g == 0:
                    xt6 = load_xt(6)
                    xt7 = load_xt(7)
                cg = (
                    iter(cproj_half(*divmod(g - 2, 2), act_ocs=(4,)))
                    if g >= 2
                    else None
                )
                for kb in range(8):
                    # steps 0-3: S^T first so its exp starts ~0.9us earlier
                    # per loop (the P@V pairs behind it keep the PE fed);
                    # steps 4-7: c_proj pieces first so a not-yet-ready S^T
                    # doesn't block them (PE executes in order).
                    if kb < 4:
                        st_step(g, kb)
                    if g >= 1 and kb < 4:
                        ot_step(g - 1, 2 * kb)
                        ot_step(g - 1, 2 * kb + 1)
                    if g >= 2 and kb == 1:
                        finalize_bcast(
                            g - 2, fz.pop(g - 2), fz.pop((g - 2, "m2"))
                        )
                    if cg is not None and kb >= 4:
                        next(cg)
                        next(cg)
                    if g == 0 and kb in (0, 1, 2):
                        qkv_chain(6, kb, xt6)
                    if g in (2, 3, 4) and kb == 2:
                        qkv_chain(7, g - 2, xt7)
                    if g == 1 and kb == 7:
                        transpose_v(6, opj)
                    if g == 5 and kb == 0:
                        transpose_v(7, opj)
                    if kb >= 4:
                        st_step(g, kb)
                    if g >= 1 and kb == 3:
                        fz[g - 1] = finalize_evict(g - 1)
                    if g >= 1 and kb == 5:
                        fz[(g - 1, "m2")] = finalize_chain(g - 1, fz[g - 1])
            # drain: P@V chain for group 7, then batch 3's two c_proj
            # halves. No exps remain, so ACT takes half the PSUM evictions
            # and the (3,0) half starts as soon as its normalize lands.
            st8 = m2row = None
            for kb in range(8):
                if kb < 4:
                    ot_step(7, 2 * kb)
                    ot_step(7, 2 * kb + 1)
                if kb == 0:
                    finalize_bcast(6, fz.pop(6), fz.pop((6, "m2")))
                if kb >= 1:
                    cg = iter(cproj_half(3, 0, (0, 2, 4, 6))) if kb == 1 else cg
                    next(cg)
                if kb == 4:
                    st8 = finalize_evict(7)
                if kb == 6:
                    m2row = finalize_chain(7, st8)
            for _ in cg:
                pass
            finalize_bcast(7, st8, m2row)
            for _ in cproj_half(3, 1, (0, 2, 4, 6)):
                pass


def _build_nc():
    nc = bacc.Bacc(
        "TRN2",
        target_bir_lowering=False,
        debug=False,
        enable_asserts=False,
        num_devices=NCORES,
    )
    xt = nc.dram_tensor("xt", [NX, T], BF16, kind="ExternalInput").ap()
    wq = nc.dram_tensor("wq", [128, 1024], BF16, kind="ExternalInput").ap()
    wk = nc.dram_tensor("wk", [128, 1024], BF16, kind="ExternalInput").ap()
    wv = nc.dram_tensor("wv", [128, 1024], BF16, kind="ExternalInput").ap()
    wp = nc.dram_tensor("wp", [128, 1024], BF16, kind="ExternalInput").ap()
    bq = nc.dram_tensor("bq", [128, 1], F32, kind="ExternalInput").ap()
    bk = nc.dram_tensor("bk", [128, 1], F32, kind="ExternalInput").ap()
    bv = nc.dram_tensor("bv", [128, 1], F32, kind="ExternalInput").ap()
    identd = nc.dram_tensor("ident", [128, 128], BF16, kind="ExternalInput").ap()
    onecd = nc.dram_tensor("onec", [128, 64], BF16, kind="ExternalInput").ap()
    onefd = nc.dram_tensor("onef", [128, 64], F32R, kind="ExternalInput").ap()
    out = nc.dram_tensor("out_t", [NX, T], F16, kind="ExternalOutput").ap()
    with tile.TileContext(nc) as tc:
        _emit(nc, tc, xt, wq, wk, wv, wp, bq, bk, bv, identd, onecd, onefd, out)
    nc.compile()
    return nc


def _pack_w(wcols):
    # [1024, 128] -> [128, 8*128] bf16: sbuf[p, k*128 + j] = W[k*128 + p, j]
    w = np.ascontiguousarray(np.asarray(wcols, dtype=np.float32))
    return np.ascontiguousarray(
        w.reshape(8, 128, 128).transpose(1, 0, 2).reshape(128, 1024)
    ).astype(ml_dtypes.bfloat16)


def kernel(hidden_states, c_attn_w, c_attn_b, c_proj_w, c_proj_b):
    global _nc_cache
    hidden_states = np.asarray(hidden_states, dtype=np.float32)
    c_attn_w = np.asarray(c_attn_w, dtype=np.float32)
    c_attn_b = np.asarray(c_attn_b, dtype=np.float32)
    c_proj_w = np.asarray(c_proj_w, dtype=np.float32)
    c_proj_b = np.asarray(c_proj_b, dtype=np.float32)

    if _nc_cache is None:
        _nc_cache = _build_nc()
    nc = _nc_cache

    X = hidden_states.reshape(T, NX)
    xt_np = np.ascontiguousarray(X.T).astype(ml_dtypes.bfloat16)

    in_maps = []
    for c in range(NCORES):
        cs = slice(c * 128, (c + 1) * 128)
        in_maps.append(
            {
                "xt": xt_np,
                "wq": _pack_w(c_attn_w[:, c * 128 : (c + 1) * 128]),
                "wk": _pack_w(c_attn_w[:, 1024 + c * 128 : 1024 + (c + 1) * 128]),
                "wv": _pack_w(c_attn_w[:, 2048 + c * 128 : 2048 + (c + 1) * 128]),
                "wp": np.ascontiguousarray(c_proj_w[cs, :]).astype(ml_dtypes.bfloat16),
                "bq": np.ascontiguousarray(c_attn_b[cs].reshape(128, 1)),
                "bk": np.ascontiguousarray(
                    c_attn_b[1024 + c * 128 : 1024 + (c + 1) * 128].reshape(128, 1)
                ),
                "bv": np.ascontiguousarray(
                    c_attn_b[2048 + c * 128 : 2048 + (c + 1) * 128].reshape(128, 1)
                ),
                "ident": np.eye(128, dtype=np.float32).astype(ml_dtypes.bfloat16),
                "onec": np.ones((128, 64), dtype=ml_dtypes.bfloat16),
                "onef": np.full((128, 64), -1.0, dtype=np.float32),
            }
        )

    trace = bool(int(os.environ.get("KERNEL_PROFILE", "0")))
    if trace:
        trace = _ensure_ntff_hook()
    try:
        res = run_bass_kernel_spmd(
            nc, in_maps, core_ids=list(range(NCORES)), trace=trace
        )
    except Exception:
        if not trace:
            raise
        print("kernel.py: traced run failed; retrying untraced")
        res = run_bass_kernel_spmd(nc, in_maps, core_ids=list(range(NCORES)))

    total = np.zeros((NX, T), np.float32)
    for r in res.results:
        total += r["out_t"].astype(np.float32)
    out = total.T.reshape(B, S, NX) + c_proj_b[None, None, :]
    kernel.last_exec_time_ns = res.exec_time_ns
    return out.astype(np.float32)
